# revision 1
# baseline (speedup 1.0000x reference)
"""Trainium2 Bass kernel for relative-position multi-head attention.

Problem: B=8, N=1024, DIM=512, H=8, DH=64, MAX_POS=512
  out = softmax(q k^T * s + pos) v @ Wo + bo,  pos[i,r] = q_i . E[clip(i-r)+512] * s

Sharding: data-parallel over batch, one batch element per NeuronCore (8 cores).

Per-core algorithm (transposed layouts, bf16 matmuls, f32 PSUM accum):
  qT/kT  = (Wq*s)^T x^T, Wk^T x^T          (inner, N)
  va     = [x Wv | ones]                   (N, 65 per head) - PV lhsT + Z row
  Gxr[i,u] = q_i . E[clip(639-u)+512]      (reversed q.E table, per head)
     -> DRAM with row pitch 1281 so each banded 128-chunk of pos^T is a
        256B-aligned row: row(i, r_b) at idx = 10*i + 1 + i//128
  dma_gather(transpose=True): g[rr, q, i] = pos^T[r, i] for r_b = i_b-4+q
  S^T(r_b) psum = k_b^T q  + identity-matmul accumulate of g slices (banded)
  + ones x Gsat rank-1 terms (saturated |i-r| >= 512 ranges)
  exp on ScalarE (PSUM -> SBUF bf16); O^T accumulated with ones-augmented V
  (row 64 = softmax denominator Z); deferred normalize by 1/Z (approx recip);
  out^T = Wo^T O^T + bo.  Host transposes back.

The Gxr build + gather for head h+1 is emitted before attention of head h
(software pipelining) so the gather latency hides under attention matmuls.
"""

import numpy as np
import ml_dtypes
import sys

sys.path.insert(0, "/opt/trn_rl_repo")

import concourse.bass as bass  # noqa: E402
import concourse.mybir as mybir  # noqa: E402
import concourse.tile as tile  # noqa: E402
from concourse import bacc  # noqa: E402
from concourse.bass_utils import run_bass_kernel_spmd  # noqa: E402

B, N, DIM = 8, 1024, 512
H, DH = 8, 64
MAX_POS = 512
SCALE = DH ** -0.5
NB = N // 128          # 8 seq blocks
WGX = 1281             # padded Etxr width: [pad | 1279 cols | pad]
PITCH = 1281           # Gxr DRAM row pitch (elements)
NROWS = 10240          # gather-view rows of 128 elems (idx max 10238)
ESIZE = 9 * 128        # gather row: 9 blocks of 128

bf16 = ml_dtypes.bfloat16
BF = mybir.dt.bfloat16
F32 = mybir.dt.float32
I16 = mybir.dt.int16


def _ap(base, rel_off, pattern):
    """Custom access pattern relative to a tile's base AP."""
    b = base[:]
    return bass.AP(tensor=b.tensor, offset=b.offset + rel_off, ap=pattern)


def build_bass(wide_band=True, big_write=True, recip_mode="exact"):
    nc = bacc.Bacc()

    xT = nc.declare_dram_parameter("xT", [DIM, N], BF, isOutput=False)
    wq = nc.declare_dram_parameter("wq", [DIM, DIM], BF, isOutput=False)
    wk = nc.declare_dram_parameter("wk", [DIM, DIM], BF, isOutput=False)
    wv = nc.declare_dram_parameter("wv", [DIM, DIM], BF, isOutput=False)
    wo = nc.declare_dram_parameter("wo", [DIM, DIM], BF, isOutput=False)
    bo = nc.declare_dram_parameter("bo", [128, 4], F32, isOutput=False)
    etxr = nc.declare_dram_parameter("etxr", [128, WGX], BF, isOutput=False)
    et2 = nc.declare_dram_parameter("et2", [128, 33], BF, isOutput=False)
    iden = nc.declare_dram_parameter("iden", [128, 128], BF, isOutput=False)
    idxs = nc.declare_dram_parameter("idxs", [128, N // 16], I16, isOutput=False)
    out = nc.declare_dram_parameter("out", [DIM, N], F32, isOutput=True)

    with tile.TileContext(nc) as tc, tc.tile_pool(name="consts", bufs=1) as consts, \
            tc.tile_pool(name="qk", bufs=1) as qkpool, \
            tc.tile_pool(name="dram", bufs=2, space="DRAM") as drampool:

        # ---------- load constants ----------
        xT_sb = [consts.tile([128, N], BF, tag=f"xt{i}", name=f"xt{i}")
                 for i in range(4)]
        wq_sb = [consts.tile([128, DIM], BF, tag=f"wq{i}", name=f"wq{i}")
                 for i in range(4)]
        wk_sb = [consts.tile([128, DIM], BF, tag=f"wk{i}", name=f"wk{i}")
                 for i in range(4)]
        wv_sb = [consts.tile([128, DIM], BF, tag=f"wv{i}", name=f"wv{i}")
                 for i in range(4)]
        wo_sb = [consts.tile([128, DIM], BF, tag=f"wo{i}", name=f"wo{i}")
                 for i in range(4)]
        for k in range(4):
            sl = slice(128 * k, 128 * k + 128)
            nc.sync.dma_start(out=xT_sb[k], in_=xT[sl, :])
            nc.sync.dma_start(out=wq_sb[k], in_=wq[sl, :])
            nc.sync.dma_start(out=wk_sb[k], in_=wk[sl, :])
            nc.sync.dma_start(out=wv_sb[k], in_=wv[sl, :])
            nc.sync.dma_start(out=wo_sb[k], in_=wo[sl, :])
        etxr_sb = consts.tile([128, WGX], BF)
        nc.sync.dma_start(out=etxr_sb, in_=etxr[:, :])
        et2_sb = consts.tile([128, 33], BF)
        nc.sync.dma_start(out=et2_sb, in_=et2[:, :])
        iden_sb = consts.tile([128, 128], BF)
        nc.sync.dma_start(out=iden_sb, in_=iden[:, :])
        idxs_sb = consts.tile([128, N // 16], I16)
        nc.sync.dma_start(out=idxs_sb, in_=idxs[:, :])
        bo_sb = consts.tile([128, 4], F32)
        nc.sync.dma_start(out=bo_sb, in_=bo[:, :])
        ones_sb = consts.tile([1, 128], BF)
        nc.vector.memset(ones_sb, 1.0)

        # long-lived activations
        qT_sb = [qkpool.tile([128, N], BF, tag=f"qt{i}", name=f"qt{i}")
                 for i in range(4)]
        kT_sb = [qkpool.tile([128, N], BF, tag=f"kt{i}", name=f"kt{i}")
                 for i in range(4)]
        va_sb = [qkpool.tile([128, H * 65], BF, tag=f"va{i}", name=f"va{i}")
                 for i in range(NB)]
        oT_sb = [qkpool.tile([128, N], BF, tag=f"ot{i}", name=f"ot{i}")
                 for i in range(4)]

        # ---------- projections ----------
        with tc.tile_pool(name="proj_psum", bufs=2, space="PSUM") as pp:
            for m in range(4):
                for c in range(2):
                    csl = slice(512 * c, 512 * c + 512)
                    pq = pp.tile([128, 512], F32, tag="pq")
                    pk = pp.tile([128, 512], F32, tag="pk")
                    for k in range(4):
                        msl = slice(128 * m, 128 * m + 128)
                        nc.tensor.matmul(pq, wq_sb[k][:, msl], xT_sb[k][:, csl],
                                         start=(k == 0), stop=(k == 3))
                        nc.tensor.matmul(pk, wk_sb[k][:, msl], xT_sb[k][:, csl],
                                         start=(k == 0), stop=(k == 3))
                    nc.scalar.copy(out=qT_sb[m][:, csl], in_=pq)
                    nc.scalar.copy(out=kT_sb[m][:, csl], in_=pk)
            for nt in range(NB):
                pv = pp.tile([128, 512], F32, tag="pv")
                for k in range(4):
                    nsl = slice(128 * nt, 128 * nt + 128)
                    nc.tensor.matmul(pv, xT_sb[k][:, nsl], wv_sb[k],
                                     start=(k == 0), stop=(k == 3))
                vout = _ap(va_sb[nt], 0, [[H * 65, 128], [65, H], [1, 64]])
                vin = _ap(pv, 0, [[512, 128], [64, H], [1, 64]])
                nc.vector.tensor_copy(vout, vin)
                oc = _ap(va_sb[nt], 64, [[H * 65, 128], [65, H], [1, 1]])
                nc.vector.memset(oc, 1.0)

        # ---------- attention ----------
        with tc.tile_pool(name="gx_psum", bufs=2, space="PSUM") as gxp, \
                tc.tile_pool(name="st_psum", bufs=3, space="PSUM") as stp, \
                tc.tile_pool(name="o_psum", bufs=1, space="PSUM") as op, \
                tc.tile_pool(name="gs_psum", bufs=1, space="PSUM") as gsp, \
                tc.tile_pool(name="gxstage", bufs=2) as gxs, \
                tc.tile_pool(name="gpool", bufs=2) as gpool, \
                tc.tile_pool(name="es", bufs=3) as esp, \
                tc.tile_pool(name="small", bufs=2) as small:

            gsat = {}
            gtiles = {}
            zz = {}

            def emit_gx(h):
                """Gxr table -> DRAM -> transposing gather, plus Gsat rows."""
                pair, off = h // 2, 64 * (h % 2)
                hsl = slice(off, off + 64)
                qTh = qT_sb[pair]
                stg = gxs.tile([128, NB, WGX], BF, tag="stg", name=f"stg{h}")
                for ib in range(NB):
                    isl = slice(128 * ib, 128 * ib + 128)
                    c0 = 0
                    while c0 < WGX:
                        cw = min(512, WGX - c0)
                        pg = gxp.tile([128, 512], F32, tag="pg",
                                      name=f"pg{h}_{ib}_{c0}")
                        nc.tensor.matmul(pg[:, :cw], qTh[hsl, isl],
                                         etxr_sb[hsl, c0:c0 + cw],
                                         start=True, stop=True)
                        if (c0 // 512 + ib) % 2 == 0:
                            nc.scalar.copy(out=stg[:, ib, c0:c0 + cw],
                                           in_=pg[:, :cw])
                        else:
                            nc.vector.tensor_copy(stg[:, ib, c0:c0 + cw],
                                                  pg[:, :cw])
                        c0 += cw
                gxr = drampool.tile([N * PITCH], BF, name=f"gxr{h}")
                if big_write:
                    # one DMA per head: (partition, ib, elem) on both sides
                    dst = _ap(gxr, 0, [[PITCH, 128], [128 * PITCH, NB], [1, WGX]])
                    nc.sync.dma_start(out=dst, in_=stg[:])
                else:
                    for ib in range(NB):
                        dst = _ap(gxr, 128 * ib * PITCH, [[PITCH, 128], [1, WGX]])
                        nc.sync.dma_start(out=dst, in_=stg[:, ib, :])
                # Gsat rows: q.E[0] (row 0 / "lo"), q.E[1024] (row 1 / "hi")
                g0 = small.tile([1, N], BF, tag="gsat0", name=f"gsat0_{h}")
                g1 = small.tile([1, N], BF, tag="gsat1", name=f"gsat1_{h}")
                for c in range(2):
                    csl = slice(512 * c, 512 * c + 512)
                    # lhsT col 0 = E[0], col 32 = E[1024] (rest zero) so both
                    # result rows land on 32-aligned PSUM partitions.
                    ps = gsp.tile([33, 512], F32, tag="ps", name=f"ps{h}_{c}")
                    nc.tensor.matmul(ps, et2_sb[hsl, :], qTh[hsl, csl],
                                     start=True, stop=True)
                    nc.vector.tensor_copy(g0[:, csl], ps[0:1, :])
                    nc.vector.tensor_copy(g1[:, csl], ps[32:33, :])
                gsat[h] = (g0, g1)
                g = gpool.tile([128, 9, N], BF, tag="g", name=f"g{h}")
                src = _ap(gxr, 0, [[128, NROWS], [1, ESIZE]])
                nc.gpsimd.dma_gather(
                    out_ap=g[:], in_ap=src, idxs_ap=idxs_sb[:],
                    num_idxs=N, num_idxs_reg=N, elem_size=ESIZE, elem_step=128,
                    transpose=True, single_packet=False,
                )
                gtiles[h] = g

            emit_gx(0)
            for h in range(H):
                if h + 1 < H:
                    emit_gx(h + 1)
                pair, off = h // 2, 64 * (h % 2)
                hsl = slice(off, off + 64)
                qTh = qT_sb[pair]
                kTh = kT_sb[pair]
                g = gtiles.pop(h)
                g0, g1 = gsat.pop(h)

                oacc = [op.tile([65, 512], F32, tag=f"oacc{c}",
                                name=f"oacc{h}_{c}") for c in range(2)]
                for rb in range(NB):
                    rsl = slice(128 * rb, 128 * rb + 128)
                    ib_lo, ib_hi = max(0, rb - 4), min(NB, rb + 5)
                    lo_end = 128 * max(0, rb - 4)    # sat-low: i < lo_end
                    hi_st = 128 * min(NB, rb + 5)    # sat-high: i >= hi_st
                    es = esp.tile([128, N], BF, tag="es", name=f"es{h}_{rb}")
                    for c in range(2):
                        c0, c1 = 512 * c, 512 * c + 512
                        pst = stp.tile([128, 512], F32, tag="pst",
                                       name=f"pst{h}_{rb}_{c}")
                        # accumulation group: main k^T q first (start=True:
                        # clears bank, sets has_written), then one wide
                        # banded identity-matmul + sat rank-1 matmuls.
                        bb_lo = max(ib_lo * 128, c0)
                        bb_hi = min(ib_hi * 128, c1)
                        nbi0 = (bb_hi - bb_lo) // 128 if bb_lo < bb_hi else 0
                        nsub = ((1 if wide_band else nbi0) if nbi0 else 0) + \
                               (1 if lo_end > c0 else 0) + (1 if hi_st < c1 else 0)
                        nc.tensor.matmul(pst, kTh[hsl, rsl], qTh[hsl, c0:c1],
                                         start=True, stop=(nsub == 0))
                        j = 0
                        if bb_lo < bb_hi:
                            nbi = (bb_hi - bb_lo) // 128
                            ib0 = bb_lo // 128
                            q0 = rb - ib0 + 4
                            if wide_band:
                                rhs = _ap(g, q0 * N + bb_lo,
                                          [[9 * N, 128], [128 - N, nbi], [1, 128]])
                                j += 1
                                nc.tensor.matmul(pst[:, bb_lo - c0:bb_hi - c0],
                                                 iden_sb, rhs,
                                                 start=False, stop=(j == nsub))
                            else:
                                for t in range(nbi):
                                    a0 = bb_lo + 128 * t
                                    j += 1
                                    nc.tensor.matmul(
                                        pst[:, a0 - c0:a0 - c0 + 128], iden_sb,
                                        g[:, q0 - t, a0:a0 + 128],
                                        start=False, stop=(j == nsub))
                        if lo_end > c0:
                            a, bnd = c0, min(lo_end, c1)
                            j += 1
                            nc.tensor.matmul(pst[:, a - c0:bnd - c0], ones_sb,
                                             g0[:, a:bnd],
                                             start=False, stop=(j == nsub))
                        if hi_st < c1:
                            a, bnd = max(hi_st, c0), c1
                            j += 1
                            nc.tensor.matmul(pst[:, a - c0:bnd - c0], ones_sb,
                                             g1[:, a:bnd],
                                             start=False, stop=(j == nsub))
                        nc.scalar.activation(es[:, c0:c1], pst,
                                             mybir.ActivationFunctionType.Exp)
                        nc.tensor.matmul(oacc[c], va_sb[rb][:, 65 * h:65 * h + 65],
                                         es[:, c0:c1],
                                         start=(rb == 0), stop=(rb == NB - 1))

                # deferred normalization bookkeeping: stash Z rows + raw O^T
                for c in range(2):
                    csl = slice(512 * c, 512 * c + 512)
                    z = small.tile([1, 512], BF, tag=f"zz{h}_{c}",
                                   name=f"zz{h}_{c}")
                    nc.vector.tensor_copy(z, oacc[c][64:65, :])
                    zz[(h, c)] = z
                    nc.scalar.copy(out=oT_sb[pair][hsl, csl],
                                   in_=oacc[c][0:64, :])

            # ---------- normalization (batched, off the critical path) ----
            for h in range(H):
                pair, off = h // 2, 64 * (h % 2)
                hsl = slice(off, off + 64)
                for c in range(2):
                    csl = slice(512 * c, 512 * c + 512)
                    bz = gxp.tile([64, 512], F32, tag="pg", name=f"bz{h}_{c}")
                    nc.tensor.matmul(bz, ones_sb[:, :64], zz[(h, c)],
                                     start=True, stop=True)
                    rz = small.tile([128, 512], F32, tag="rz", name=f"rz{h}_{c}")
                    if recip_mode == "approx_psum":
                        nc.vector.reciprocal_approx_fast(out=rz[hsl, :], in_=bz)
                    elif recip_mode == "approx_sbuf":
                        zs = small.tile([128, 512], F32, tag="zs",
                                        name=f"zs{h}_{c}")
                        nc.scalar.copy(out=zs[hsl, :], in_=bz)
                        nc.vector.reciprocal_approx_fast(out=rz[hsl, :],
                                                         in_=zs[hsl, :])
                    else:
                        nc.vector.reciprocal(rz[hsl, :], bz)
                    nc.vector.tensor_mul(oT_sb[pair][hsl, csl],
                                         oT_sb[pair][hsl, csl], rz[hsl, :])

        # ---------- output projection ----------
        with tc.tile_pool(name="oproj_psum", bufs=4, space="PSUM") as opp, \
                tc.tile_pool(name="osb", bufs=4) as osb:
            for m in range(4):
                msl = slice(128 * m, 128 * m + 128)
                for c in range(2):
                    csl = slice(512 * c, 512 * c + 512)
                    po = opp.tile([128, 512], F32, tag="po")
                    for k in range(4):
                        nc.tensor.matmul(po, wo_sb[k][:, msl], oT_sb[k][:, csl],
                                         start=(k == 0), stop=(k == 3))
                    ot = osb.tile([128, 512], F32, tag="otf")
                    nc.scalar.add(out=ot, in_=po, add=bo_sb[:, m:m + 1])
                    nc.sync.dma_start(out=out[msl, csl], in_=ot)
    nc.compile()
    return nc


_NC_CACHE = {}


def _get_nc():
    if "nc" not in _NC_CACHE:
        _NC_CACHE["nc"] = build_bass()
    return _NC_CACHE["nc"]


def _host_prep(x, Wq, Wkv, Wo, bo, E):
    u = np.clip(639 - (np.arange(WGX) - 1), -512, 512) + 512
    etxr = E[u].T.astype(bf16)                                   # (64, WGX)
    etxr = np.concatenate([etxr, etxr], axis=0)                  # dup rows
    et2 = np.zeros((DH, 33), bf16)                               # (64, 33)
    et2[:, 0] = E[0].astype(bf16)
    et2[:, 32] = E[2 * MAX_POS].astype(bf16)
    et2 = np.concatenate([et2, et2], axis=0)
    ii = np.arange(N)
    idx = (10 * ii + 1 + ii // 128).astype(np.int16)
    idxs = np.zeros((16, N // 16), np.int16)
    idxs[ii % 16, ii // 16] = idx
    idxs = np.tile(idxs, (8, 1))                                 # (128, 64)
    common = {
        "wq": (Wq * SCALE).astype(bf16),
        "wk": Wkv[:, :DIM].astype(bf16),
        "wv": Wkv[:, DIM:].astype(bf16),
        "wo": Wo.astype(bf16),
        "bo": np.ascontiguousarray(bo.reshape(4, 128).T.astype(np.float32)),
        "etxr": np.ascontiguousarray(etxr),
        "et2": np.ascontiguousarray(et2),
        "iden": np.eye(128, dtype=bf16),
        "idxs": idxs,
    }
    in_maps = []
    for b in range(B):
        m = dict(common)
        m["xT"] = np.ascontiguousarray(x[b].T.astype(bf16))
        in_maps.append(m)
    return in_maps


def kernel(x, Wq, Wkv, Wo, bo, E):
    x, Wq, Wkv, Wo, bo, E = (np.asarray(a) for a in (x, Wq, Wkv, Wo, bo, E))
    nc = _get_nc()
    in_maps = _host_prep(x, Wq, Wkv, Wo, bo, E)
    res = run_bass_kernel_spmd(nc, in_maps, core_ids=list(range(B)))
    out = np.stack([np.asarray(res.results[b]["out"], dtype=np.float32).T
                    for b in range(B)])
    return out


if __name__ == "__main__":
    rng = np.random.default_rng(0)
    inputs = {
        "x": rng.standard_normal((B, N, DIM), dtype=np.float32),
        "Wq": rng.standard_normal((DIM, H * DH), dtype=np.float32) * DIM ** -0.5,
        "Wkv": rng.standard_normal((DIM, 2 * H * DH), dtype=np.float32) * DIM ** -0.5,
        "Wo": rng.standard_normal((H * DH, DIM), dtype=np.float32) * (H * DH) ** -0.5,
        "bo": np.zeros((DIM,), np.float32),
        "E": rng.standard_normal((2 * MAX_POS + 1, DH), dtype=np.float32),
    }
    o = kernel(**inputs)
    print("kernel ran, out shape", o.shape, "sample", o[0, 0, :4])



# revision 9
# speedup vs baseline: 1.1014x; 1.1014x over previous
"""Trainium2 Bass kernel for relative-position multi-head attention.

Problem: B=8, N=1024, DIM=512, H=8, DH=64, MAX_POS=512
  out = softmax(q k^T * s + pos) v @ Wo + bo,  pos[i,r] = q_i . E[clip(i-r)+512] * s

Sharding: data-parallel over batch, one batch element per NeuronCore (8 cores).

Per-core algorithm (transposed layouts, bf16 matmuls, f32 PSUM accum), v2:
  qT/kT  = (Wq*s)^T x^T, Wk^T x^T          (inner, N)
  va     = [x Wv | ones]                   (N, 65 per head) - PV lhsT + Z row
  gsat   = (x Wsat)^T                      (16, N) rows 2h/2h+1 = q_h.E[0|1024]
  Gxr[i,u] = q_i . E[clip(639-u)+512]      (reversed q.E table, per head,
     valid-band columns only) -> DRAM with row pitch 1281 so each banded
     128-chunk of pos^T is a 256B-aligned row
  dma_gather(transpose=True) per (head, query-half): g[rr, q, i] = pos^T[r, i]
  S^T(rb) psum = k_b^T q + wide banded identity-matmul + K=16 selector
  matmuls against gsat for saturated ranges; exp on ScalarE -> bf16 SBUF;
  O^T accumulated with ones-augmented V (row 64 = Z); deferred per-pair
  normalize by approx-reciprocal; out^T = Wo^T O^T + bo. Host transposes.

Heads are processed in PAIRS (2p, 2p+1): their q/k/E data live at SBUF
partitions 0-63 / 64-127, so the K=64 matmuls of the two heads target
disjoint PE row-groups and execute concurrently (auto row-tiling).
The Gxr build + gather for pair p+1 is interleaved chunk-by-chunk into
attention of pair p; normalization of pair p runs during pair p+1.
"""

import numpy as np
import ml_dtypes
import sys

sys.path.insert(0, "/opt/trn_rl_repo")

import concourse.bass as bass  # noqa: E402
import concourse.mybir as mybir  # noqa: E402
import concourse.tile as tile  # noqa: E402
from concourse import bacc  # noqa: E402
from concourse.bass_utils import run_bass_kernel_spmd  # noqa: E402

B, N, DIM = 8, 1024, 512
H, DH = 8, 64
MAX_POS = 512
SCALE = DH ** -0.5
NB = N // 128          # 8 seq blocks
WGX = 1281             # padded Etxr width: [pad | 1279 cols | pad]
PITCH = 1281           # Gxr DRAM row pitch (elements)
ESIZE = 9 * 128        # gather row: 9 blocks of 128
HALF = N // 2          # queries per gather half
HROWS = (HALF * PITCH - ESIZE) // 128 + 1  # in-bounds gather view rows (5116)
IDX_REBASE = 5124      # row offset of query 512's data in its half tile

bf16 = ml_dtypes.bfloat16
BF = mybir.dt.bfloat16
F32 = mybir.dt.float32
I16 = mybir.dt.int16


def _ap(base, rel_off, pattern):
    """Custom access pattern relative to a tile's base AP."""
    b = base[:]
    return bass.AP(tensor=b.tensor, offset=b.offset + rel_off, ap=pattern)


def _valid_cols(ib):
    """Valid Gxr column range [c_lo, c_hi) of the WGX table for query block ib."""
    lo_q = max(0, 4 - ib)
    hi_q = min(8, 11 - ib)
    return 1 + 128 * lo_q, 128 * (hi_q + 2)


def build_bass():
    nc = bacc.Bacc()

    xT = nc.declare_dram_parameter("xT", [DIM, N], BF, isOutput=False)
    wq = nc.declare_dram_parameter("wq", [DIM, DIM], BF, isOutput=False)
    wk = nc.declare_dram_parameter("wk", [DIM, DIM], BF, isOutput=False)
    wv = nc.declare_dram_parameter("wv", [DIM, DIM], BF, isOutput=False)
    wo = nc.declare_dram_parameter("wo", [DIM, DIM], BF, isOutput=False)
    bo = nc.declare_dram_parameter("bo", [128, 4], F32, isOutput=False)
    etxr = nc.declare_dram_parameter("etxr", [128, WGX], BF, isOutput=False)
    wsat = nc.declare_dram_parameter("wsat", [DIM, 16], BF, isOutput=False)
    sel = nc.declare_dram_parameter("sel", [16, 16 * 128], BF, isOutput=False)
    iden = nc.declare_dram_parameter("iden", [128, 128], BF, isOutput=False)
    idxs = nc.declare_dram_parameter("idxs", [128, N // 16], I16, isOutput=False)
    out = nc.declare_dram_parameter("out", [DIM, N], F32, isOutput=True)

    with tile.TileContext(nc) as tc, tc.tile_pool(name="consts", bufs=1) as consts, \
            tc.tile_pool(name="qk", bufs=1) as qkpool, \
            tc.tile_pool(name="dram", bufs=4, space="DRAM") as drampool:

        # ---------- load constants ----------
        xT_sb = [consts.tile([128, N], BF, tag=f"xt{i}", name=f"xt{i}")
                 for i in range(4)]
        wq_sb = [consts.tile([128, DIM], BF, tag=f"wq{i}", name=f"wq{i}")
                 for i in range(4)]
        wk_sb = [consts.tile([128, DIM], BF, tag=f"wk{i}", name=f"wk{i}")
                 for i in range(4)]
        wv_sb = [consts.tile([128, DIM], BF, tag=f"wv{i}", name=f"wv{i}")
                 for i in range(4)]
        wo_sb = [consts.tile([128, DIM], BF, tag=f"wo{i}", name=f"wo{i}")
                 for i in range(4)]
        ws_sb = [consts.tile([128, 16], BF, tag=f"ws{i}", name=f"ws{i}")
                 for i in range(4)]
        for k in range(4):
            sl = slice(128 * k, 128 * k + 128)
            nc.sync.dma_start(out=xT_sb[k], in_=xT[sl, :])
            nc.sync.dma_start(out=wq_sb[k], in_=wq[sl, :])
            nc.sync.dma_start(out=wk_sb[k], in_=wk[sl, :])
            nc.sync.dma_start(out=wv_sb[k], in_=wv[sl, :])
            nc.sync.dma_start(out=wo_sb[k], in_=wo[sl, :])
            nc.sync.dma_start(out=ws_sb[k], in_=wsat[sl, :])
        etxr_sb = consts.tile([128, WGX], BF)
        nc.sync.dma_start(out=etxr_sb, in_=etxr[:, :])
        sel_sb = consts.tile([16, 16 * 128], BF)
        nc.sync.dma_start(out=sel_sb, in_=sel[:, :])
        iden_sb = consts.tile([128, 128], BF)
        nc.sync.dma_start(out=iden_sb, in_=iden[:, :])
        idxs_sb = consts.tile([128, N // 16], I16)
        nc.sync.dma_start(out=idxs_sb, in_=idxs[:, :])
        bo_sb = consts.tile([128, 4], F32)
        nc.sync.dma_start(out=bo_sb, in_=bo[:, :])
        ones_sb = consts.tile([1, 128], BF)
        nc.vector.memset(ones_sb, 1.0)

        # long-lived activations
        qT_sb = [qkpool.tile([128, N], BF, tag=f"qt{i}", name=f"qt{i}")
                 for i in range(4)]
        kT_sb = [qkpool.tile([128, N], BF, tag=f"kt{i}", name=f"kt{i}")
                 for i in range(4)]
        va_sb = [qkpool.tile([128, H * 65], BF, tag=f"va{i}", name=f"va{i}")
                 for i in range(NB)]
        oT_sb = [qkpool.tile([128, N], BF, tag=f"ot{i}", name=f"ot{i}")
                 for i in range(4)]
        gsat_sb = qkpool.tile([16, N], BF, tag="gsat", name="gsat")
        warm_sb = qkpool.tile([1, 8], BF, tag="warm", name="warm")

        # pre-warm the exp activation table (one tiny call; ~2.7us table load)
        nc.scalar.activation(warm_sb[:, 0:4], bo_sb[0:1, 0:4],
                             mybir.ActivationFunctionType.Exp)

        # ---------- q/k/gsat projections ----------
        with tc.tile_pool(name="proj_psum", bufs=2, space="PSUM") as pp:
            for m in range(4):
                for c in range(2):
                    csl = slice(512 * c, 512 * c + 512)
                    pq = pp.tile([128, 512], F32, tag="pq")
                    pk = pp.tile([128, 512], F32, tag="pk")
                    for k in range(4):
                        msl = slice(128 * m, 128 * m + 128)
                        nc.tensor.matmul(pq, wq_sb[k][:, msl], xT_sb[k][:, csl],
                                         start=(k == 0), stop=(k == 3))
                        nc.tensor.matmul(pk, wk_sb[k][:, msl], xT_sb[k][:, csl],
                                         start=(k == 0), stop=(k == 3))
                    nc.scalar.copy(out=qT_sb[m][:, csl], in_=pq)
                    nc.scalar.copy(out=kT_sb[m][:, csl], in_=pk)
            for c in range(2):
                csl = slice(512 * c, 512 * c + 512)
                psg = pp.tile([16, 512], F32, tag="psg")
                for k in range(4):
                    nc.tensor.matmul(psg, ws_sb[k], xT_sb[k][:, csl],
                                     start=(k == 0), stop=(k == 3))
                nc.scalar.copy(out=gsat_sb[:, csl], in_=psg)

        # ---------- attention (+ v-projection interleaved with Gxr pair 0) ----
        with tc.tile_pool(name="pg_psum", bufs=2, space="PSUM") as pgp, \
                tc.tile_pool(name="st_psum", bufs=2, space="PSUM") as stp, \
                tc.tile_pool(name="o_psum", bufs=2, space="PSUM") as op, \
                tc.tile_pool(name="gxstage", bufs=1) as gxs, \
                tc.tile_pool(name="gpool", bufs=8) as gpool, \
                tc.tile_pool(name="es", bufs=2) as esp, \
                tc.tile_pool(name="small", bufs=2) as small:

            gtiles = {}    # (h, c) -> [128, 9, HALF] gathered pos^T bands

            def gx_steps(p, evac_cnt=[0]):
                """Generator of emission steps building pair p's Gxr tables.

                Each step: one 512-col chunk for BOTH heads (concurrent
                K=64 row-tiles) + PSUM evacuation; write DMAs fire per
                (head, ib); gathers per (head, query-half)."""
                h0 = 2 * p
                qTh = qT_sb[p]
                stg = {hx: gxs.tile([128, NB, WGX], BF, tag=f"stg{hx}",
                                    name=f"stg{2 * p + hx}") for hx in (0, 1)}
                gxr = {}
                for half in range(2):
                    for hx in (0, 1):
                        gxr[(hx, half)] = drampool.tile(
                            [HALF * PITCH], BF, tag="gx",
                            name=f"gxr{2 * p + hx}_{half}")

                for ib in range(NB):
                    isl = slice(128 * ib, 128 * ib + 128)
                    c_lo, c_hi = _valid_cols(ib)
                    chunks = list(range(c_lo, c_hi, 512))
                    for ci, c0 in enumerate(chunks):
                        cw = min(512, c_hi - c0)
                        last = ci == len(chunks) - 1

                        def step(ib=ib, isl=isl, c0=c0, cw=cw, last=last,
                                 c_lo=c_lo, c_hi=c_hi):
                            for hx in (0, 1):
                                hsl = slice(64 * hx, 64 * hx + 64)
                                pg = pgp.tile([128, 512], F32, tag="pg",
                                              name=f"pg{p}_{hx}_{ib}_{c0}")
                                nc.tensor.matmul(pg[:, :cw], qTh[hsl, isl],
                                                 etxr_sb[hsl, c0:c0 + cw],
                                                 start=True, stop=True)
                                evac_cnt[0] += 1
                                eng = (nc.scalar if evac_cnt[0] % 16 == 15
                                       else nc.vector)
                                if eng is nc.scalar:
                                    nc.scalar.copy(
                                        out=stg[hx][:, ib, c0:c0 + cw],
                                        in_=pg[:, :cw])
                                else:
                                    nc.vector.tensor_copy(
                                        stg[hx][:, ib, c0:c0 + cw],
                                        pg[:, :cw])
                            if last:
                                half = ib // 4
                                ibl = ib % 4
                                w = c_hi - c_lo
                                for hx in (0, 1):
                                    dst = _ap(gxr[(hx, half)],
                                              128 * ibl * PITCH + c_lo,
                                              [[PITCH, 128], [1, w]])
                                    nc.sync.dma_start(
                                        out=dst,
                                        in_=stg[hx][:, ib, c_lo:c_hi])
                        yield step

                    if ib % 4 == 3:
                        half = ib // 4

                        def gstep(half=half):
                            for hx in (0, 1):
                                g = gpool.tile([128, 9, HALF], BF, tag="g",
                                               name=f"g{2 * p + hx}_{half}")
                                src = _ap(gxr[(hx, half)], 0,
                                          [[128, HROWS], [1, ESIZE]])
                                nc.gpsimd.dma_gather(
                                    out_ap=g[:], in_ap=src,
                                    idxs_ap=idxs_sb[:, 32 * half:32 * half + 32],
                                    num_idxs=HALF, num_idxs_reg=HALF,
                                    elem_size=ESIZE, elem_step=128,
                                    transpose=True, single_packet=False,
                                )
                                gtiles[(2 * p + hx, half)] = g
                        yield gstep

            def v_steps():
                """v-projection + ones-augmented va build, as filler steps."""
                for nt in range(NB):
                    def step(nt=nt):
                        pv = pgp.tile([128, 512], F32, tag="pg",
                                      name=f"pv{nt}")
                        for k in range(4):
                            nsl = slice(128 * nt, 128 * nt + 128)
                            nc.tensor.matmul(pv, xT_sb[k][:, nsl], wv_sb[k],
                                             start=(k == 0), stop=(k == 3))
                        vout = _ap(va_sb[nt], 0,
                                   [[H * 65, 128], [65, H], [1, 64]])
                        vin = _ap(pv, 0, [[512, 128], [64, H], [1, 64]])
                        nc.vector.tensor_copy(vout, vin)
                        oc = _ap(va_sb[nt], 64, [[H * 65, 128], [65, H], [1, 1]])
                        nc.vector.memset(oc, 1.0)
                    yield step

            def drain(it, k=1):
                n = 0
                for s in it:
                    s()
                    n += 1
                    if n >= k:
                        return

            # interleave v-projection with pair-0 Gxr build
            gx0 = gx_steps(0)
            vp = v_steps()
            both = True
            while both:
                both = False
                for it in (gx0, vp, gx0):
                    for s in it:
                        s()
                        both = True
                        break

            zz = {}        # (h, c) -> Z row [1, 512]
            norm_q = []    # deferred normalization closures

            def make_norm(p, hx, c):
                def closure():
                    hsl = slice(64 * hx, 64 * hx + 64)
                    csl = slice(512 * c, 512 * c + 512)
                    bz = pgp.tile([128, 512], F32, tag="pg",
                                  name=f"bz{2 * p + hx}_{c}")
                    nc.tensor.matmul(bz, ones_sb,
                                     zz.pop((2 * p + hx, c)),
                                     start=True, stop=True)
                    rz = small.tile([128, 512], F32, tag=f"rz{hx}",
                                    name=f"rz{2 * p + hx}_{c}")
                    nc.vector.reciprocal_approx_fast(out=rz, in_=bz)
                    nc.vector.tensor_mul(oT_sb[p][hsl, csl],
                                         oT_sb[p][hsl, csl], rz[hsl, :])
                return closure

            def attention_pair(p, gx_iter):
                qTh = qT_sb[p]
                kTh = kT_sb[p]
                pend = None
                oacc_live = {}

                def emit_pv(c, rb, es2):
                    """PV accumulate; allocates oacc lazily at rb==0 so pool
                    rotation order matches instruction emission order."""
                    if rb == 0:
                        oacc_live[c] = {
                            hx: op.tile([65, 512], F32, tag="oacc",
                                        name=f"oacc{2 * p + hx}_{c}")
                            for hx in (0, 1)}
                    for hx in (0, 1):
                        h = 2 * p + hx
                        nc.tensor.matmul(
                            oacc_live[c][hx], va_sb[rb][:, 65 * h:65 * h + 65],
                            es2[hx], start=(rb == 0), stop=(rb == NB - 1))
                    if rb == NB - 1:
                        emit_stash(c, oacc_live.pop(c))

                def emit_stash(c, oacc2):
                    for hx in (0, 1):
                        h = 2 * p + hx
                        hsl = slice(64 * hx, 64 * hx + 64)
                        csl = slice(512 * c, 512 * c + 512)
                        z = small.tile([1, 512], BF, tag=f"zz{hx}",
                                       name=f"zz{h}_{c}")
                        nc.scalar.copy(out=z, in_=oacc2[hx][64:65, :])
                        zz[(h, c)] = z
                        nc.vector.tensor_copy(oT_sb[p][hsl, csl],
                                              oacc2[hx][0:64, :])
                        norm_q.append(make_norm(p, hx, c))

                for c in range(2):
                    c0a, c1a = 512 * c, 512 * c + 512
                    for rb in range(NB):
                        rsl = slice(128 * rb, 128 * rb + 128)
                        ib_lo, ib_hi = max(0, rb - 4), min(NB, rb + 5)
                        lo_end = 128 * max(0, rb - 4)
                        hi_st = 128 * min(NB, rb + 5)
                        bb_lo = max(ib_lo * 128, c0a)
                        bb_hi = min(ib_hi * 128, c1a)
                        nbi0 = (bb_hi - bb_lo) // 128 if bb_lo < bb_hi else 0
                        nsub = (1 if nbi0 else 0) + \
                               (1 if lo_end > c0a else 0) + \
                               (1 if hi_st < c1a else 0)

                        if pend is not None:
                            emit_pv(*pend)
                            pend = None

                        pst2 = {}
                        for hx in (0, 1):
                            h = 2 * p + hx
                            hsl = slice(64 * hx, 64 * hx + 64)
                            pst = stp.tile([128, 512], F32, tag=f"pst{hx}",
                                           name=f"pst{h}_{c}_{rb}")
                            pst2[hx] = pst
                            nc.tensor.matmul(pst, kTh[hsl, rsl],
                                             qTh[hsl, c0a:c1a],
                                             start=True, stop=(nsub == 0))
                            j = 0
                            if nbi0:
                                ib0 = bb_lo // 128
                                q0 = rb - ib0 + 4
                                g = gtiles[(h, c)]
                                rhs = _ap(g, q0 * HALF + (bb_lo - c0a),
                                          [[9 * HALF, 128],
                                           [128 - HALF, nbi0], [1, 128]])
                                j += 1
                                nc.tensor.matmul(
                                    pst[:, bb_lo - c0a:bb_hi - c0a],
                                    iden_sb, rhs,
                                    start=False, stop=(j == nsub))
                            if lo_end > c0a:
                                a, bnd = c0a, min(lo_end, c1a)
                                j += 1
                                nc.tensor.matmul(
                                    pst[:, a - c0a:bnd - c0a],
                                    sel_sb[:, 128 * 2 * h:128 * 2 * h + 128],
                                    gsat_sb[:, a:bnd],
                                    start=False, stop=(j == nsub))
                            if hi_st < c1a:
                                a, bnd = max(hi_st, c0a), c1a
                                j += 1
                                nc.tensor.matmul(
                                    pst[:, a - c0a:bnd - c0a],
                                    sel_sb[:, 128 * (2 * h + 1):
                                           128 * (2 * h + 1) + 128],
                                    gsat_sb[:, a:bnd],
                                    start=False, stop=(j == nsub))

                        if gx_iter is not None:
                            drain(gx_iter, 1)
                        if norm_q:
                            norm_q.pop(0)()

                        es2 = {}
                        for hx in (0, 1):
                            es = esp.tile([128, 512], BF, tag=f"es{hx}",
                                          name=f"es{2 * p + hx}_{c}_{rb}")
                            nc.scalar.activation(
                                es, pst2[hx],
                                mybir.ActivationFunctionType.Exp)
                            es2[hx] = es
                        pend = (c, rb, es2)

                    # free the g tiles of this (pair, c)
                    for hx in (0, 1):
                        gtiles.pop((2 * p + hx, c), None)

                # drain remaining gx steps, then flush the last PV + stash
                if gx_iter is not None:
                    for s in gx_iter:
                        s()
                emit_pv(*pend)

            for p in range(4):
                gx_iter = gx_steps(p + 1) if p + 1 < 4 else None
                attention_pair(p, gx_iter)
            while norm_q:
                norm_q.pop(0)()

        # ---------- output projection ----------
        with tc.tile_pool(name="oproj_psum", bufs=4, space="PSUM") as opp, \
                tc.tile_pool(name="osb", bufs=4) as osb:
            for m in range(4):
                msl = slice(128 * m, 128 * m + 128)
                for c in range(2):
                    csl = slice(512 * c, 512 * c + 512)
                    po = opp.tile([128, 512], F32, tag="po")
                    for k in range(4):
                        nc.tensor.matmul(po, wo_sb[k][:, msl], oT_sb[k][:, csl],
                                         start=(k == 0), stop=(k == 3))
                    ot = osb.tile([128, 512], F32, tag="otf")
                    nc.scalar.add(out=ot, in_=po, add=bo_sb[:, m:m + 1])
                    nc.sync.dma_start(out=out[msl, csl], in_=ot)
    nc.compile()
    return nc


_NC_CACHE = {}


def _get_nc():
    if "nc" not in _NC_CACHE:
        _NC_CACHE["nc"] = build_bass()
    return _NC_CACHE["nc"]


def _host_prep(x, Wq, Wkv, Wo, bo, E):
    u = np.clip(639 - (np.arange(WGX) - 1), -512, 512) + 512
    etxr = E[u].T.astype(bf16)                                   # (64, WGX)
    etxr = np.concatenate([etxr, etxr], axis=0)                  # dup rows
    Wqs = (Wq * SCALE).astype(np.float32)
    wsat = np.zeros((DIM, 16), np.float32)
    for h in range(H):
        wsat[:, 2 * h] = Wqs[:, 64 * h:64 * h + 64] @ E[0]
        wsat[:, 2 * h + 1] = Wqs[:, 64 * h:64 * h + 64] @ E[2 * MAX_POS]
    sel = np.zeros((16, 16 * 128), bf16)
    for t in range(16):
        sel[t, 128 * t:128 * t + 128] = 1.0
    ii = np.arange(N)
    idx = (10 * ii + 1 + ii // 128 - np.where(ii >= HALF, IDX_REBASE, 0))
    idx = idx.astype(np.int16)
    idxs = np.zeros((16, N // 16), np.int16)
    idxs[ii % 16, ii // 16] = idx
    idxs = np.tile(idxs, (8, 1))                                 # (128, 64)
    common = {
        "wq": Wqs.astype(bf16),
        "wk": Wkv[:, :DIM].astype(bf16),
        "wv": Wkv[:, DIM:].astype(bf16),
        "wo": Wo.astype(bf16),
        "bo": np.ascontiguousarray(bo.reshape(4, 128).T.astype(np.float32)),
        "etxr": np.ascontiguousarray(etxr),
        "wsat": wsat.astype(bf16),
        "sel": sel,
        "iden": np.eye(128, dtype=bf16),
        "idxs": idxs,
    }
    in_maps = []
    for b in range(B):
        m = dict(common)
        m["xT"] = np.ascontiguousarray(x[b].T.astype(bf16))
        in_maps.append(m)
    return in_maps


def kernel(x, Wq, Wkv, Wo, bo, E):
    x, Wq, Wkv, Wo, bo, E = (np.asarray(a) for a in (x, Wq, Wkv, Wo, bo, E))
    nc = _get_nc()
    in_maps = _host_prep(x, Wq, Wkv, Wo, bo, E)
    res = run_bass_kernel_spmd(nc, in_maps, core_ids=list(range(B)))
    out = np.stack([np.asarray(res.results[b]["out"], dtype=np.float32).T
                    for b in range(B)])
    return out


if __name__ == "__main__":
    rng = np.random.default_rng(0)
    inputs = {
        "x": rng.standard_normal((B, N, DIM), dtype=np.float32),
        "Wq": rng.standard_normal((DIM, H * DH), dtype=np.float32) * DIM ** -0.5,
        "Wkv": rng.standard_normal((DIM, 2 * H * DH), dtype=np.float32) * DIM ** -0.5,
        "Wo": rng.standard_normal((H * DH, DIM), dtype=np.float32) * (H * DH) ** -0.5,
        "bo": np.zeros((DIM,), np.float32),
        "E": rng.standard_normal((2 * MAX_POS + 1, DH), dtype=np.float32),
    }
    o = kernel(**inputs)
    print("kernel ran, out shape", o.shape, "sample", o[0, 0, :4])


# revision 16
# speedup vs baseline: 1.3015x; 1.1816x over previous
"""Trainium2 Bass kernel for relative-position multi-head attention.

Problem: B=8, N=1024, DIM=512, H=8, DH=64, MAX_POS=512
  out = softmax(q k^T * s + pos) v @ Wo + bo,  pos[i,r] = q_i . E[clip(i-r)+512] * s

Sharding: data-parallel over batch, one batch element per NeuronCore (8 cores).

Per-core algorithm (transposed layouts, bf16 matmuls, f32 PSUM accum), v2:
  qT/kT  = (Wq*s)^T x^T, Wk^T x^T          (inner, N)
  va     = [x Wv | ones]                   (N, 65 per head) - PV lhsT + Z row
  gsat   = (x Wsat)^T                      (16, N) rows 2h/2h+1 = q_h.E[0|1024]
  Gxr[i,u] = q_i . E[clip(639-u)+512]      (reversed q.E table, per head,
     valid-band columns only) -> DRAM with row pitch 1281 so each banded
     128-chunk of pos^T is a 256B-aligned row
  dma_gather(transpose=True) per (head, query-half): g[rr, q, i] = pos^T[r, i]
  S^T(rb) psum = k_b^T q + wide banded identity-matmul + K=16 selector
  matmuls against gsat for saturated ranges; exp on ScalarE -> bf16 SBUF;
  O^T accumulated with ones-augmented V (row 64 = Z); deferred per-pair
  normalize by approx-reciprocal; out^T = Wo^T O^T + bo. Host transposes.

Heads are processed in PAIRS (2p, 2p+1): their q/k/E data live at SBUF
partitions 0-63 / 64-127, so the K=64 matmuls of the two heads target
disjoint PE row-groups and execute concurrently (auto row-tiling).
The Gxr build + gather for pair p+1 is interleaved chunk-by-chunk into
attention of pair p; normalization of pair p runs during pair p+1.
"""

import numpy as np
import ml_dtypes
import sys

sys.path.insert(0, "/opt/trn_rl_repo")

import concourse.bass as bass  # noqa: E402
import concourse.mybir as mybir  # noqa: E402
import concourse.tile as tile  # noqa: E402
from concourse import bacc  # noqa: E402
from concourse.bass_utils import run_bass_kernel_spmd  # noqa: E402

B, N, DIM = 8, 1024, 512
H, DH = 8, 64
MAX_POS = 512
SCALE = DH ** -0.5
NB = N // 128          # 8 seq blocks
WGX = 1281             # padded Etxr width: [pad | 1279 cols | pad]
PITCH = 1281           # Gxr DRAM row pitch (elements)
ESIZE = 9 * 128        # gather row: 9 blocks of 128
HALF = N // 2          # queries per gather half
HROWS = (HALF * PITCH - ESIZE) // 128 + 1  # in-bounds gather view rows (5116)
IDX_REBASE = 5124      # row offset of query 512's data in its half tile

bf16 = ml_dtypes.bfloat16
BF = mybir.dt.bfloat16
F32 = mybir.dt.float32
I16 = mybir.dt.int16


def _ap(base, rel_off, pattern):
    """Custom access pattern relative to a tile's base AP."""
    b = base[:]
    return bass.AP(tensor=b.tensor, offset=b.offset + rel_off, ap=pattern)


def _valid_cols(ib):
    """Valid Gxr column range [c_lo, c_hi) of the WGX table for query block ib."""
    lo_q = max(0, 4 - ib)
    hi_q = min(8, 11 - ib)
    return 1 + 128 * lo_q, 128 * (hi_q + 2)


def build_bass():
    nc = bacc.Bacc()

    xT = nc.declare_dram_parameter("xT", [DIM, N], BF, isOutput=False)
    wq = nc.declare_dram_parameter("wq", [DIM, DIM], BF, isOutput=False)
    wk = nc.declare_dram_parameter("wk", [DIM, DIM], BF, isOutput=False)
    wv = nc.declare_dram_parameter("wv", [DIM, DIM], BF, isOutput=False)
    wo = nc.declare_dram_parameter("wo", [DIM, DIM], BF, isOutput=False)
    bo = nc.declare_dram_parameter("bo", [128, 4], F32, isOutput=False)
    etxr = nc.declare_dram_parameter("etxr", [128, WGX], BF, isOutput=False)
    wsat = nc.declare_dram_parameter("wsat", [DIM, 16], BF, isOutput=False)
    sel = nc.declare_dram_parameter("sel", [64, 16 * 128], BF, isOutput=False)
    iden = nc.declare_dram_parameter("iden", [128, 128], BF, isOutput=False)
    idxs = nc.declare_dram_parameter("idxs", [128, N // 16], I16, isOutput=False)
    out = nc.declare_dram_parameter("out", [DIM, N], F32, isOutput=True)

    with tile.TileContext(nc) as tc, tc.tile_pool(name="consts", bufs=1) as consts, \
            tc.tile_pool(name="qk", bufs=1) as qkpool, \
            tc.tile_pool(name="dram", bufs=4, space="DRAM") as drampool:

        # ---------- load constants ----------
        xT_sb = [consts.tile([128, N], BF, tag=f"xt{i}", name=f"xt{i}")
                 for i in range(4)]
        wq_sb = [consts.tile([128, DIM], BF, tag=f"wq{i}", name=f"wq{i}")
                 for i in range(4)]
        wk_sb = [consts.tile([128, DIM], BF, tag=f"wk{i}", name=f"wk{i}")
                 for i in range(4)]
        wv_sb = [consts.tile([128, DIM], BF, tag=f"wv{i}", name=f"wv{i}")
                 for i in range(4)]
        wo_sb = [consts.tile([128, DIM], BF, tag=f"wo{i}", name=f"wo{i}")
                 for i in range(4)]
        ws_sb = [consts.tile([128, 16], BF, tag=f"ws{i}", name=f"ws{i}")
                 for i in range(4)]
        for k in range(4):
            sl = slice(128 * k, 128 * k + 128)
            nc.sync.dma_start(out=xT_sb[k], in_=xT[sl, :])
            nc.sync.dma_start(out=wq_sb[k], in_=wq[sl, :])
            nc.sync.dma_start(out=wk_sb[k], in_=wk[sl, :])
            nc.sync.dma_start(out=wv_sb[k], in_=wv[sl, :])
            nc.sync.dma_start(out=wo_sb[k], in_=wo[sl, :])
            nc.sync.dma_start(out=ws_sb[k], in_=wsat[sl, :])
        etxr_sb = consts.tile([128, WGX], BF)
        nc.sync.dma_start(out=etxr_sb, in_=etxr[:, :])
        sel_sb = consts.tile([64, 16 * 128], BF)
        nc.sync.dma_start(out=sel_sb, in_=sel[:, :])
        iden_sb = consts.tile([128, 128], BF)
        nc.sync.dma_start(out=iden_sb, in_=iden[:, :])
        idxs_sb = consts.tile([128, N // 16], I16)
        nc.sync.dma_start(out=idxs_sb, in_=idxs[:, :])
        bo_sb = consts.tile([128, 4], F32)
        nc.sync.dma_start(out=bo_sb, in_=bo[:, :])
        ones_sb = consts.tile([1, 128], BF)
        nc.vector.memset(ones_sb, 1.0)

        # long-lived activations
        qT_sb = [qkpool.tile([128, N], BF, tag=f"qt{i}", name=f"qt{i}")
                 for i in range(4)]
        kT_sb = [qkpool.tile([128, N], BF, tag=f"kt{i}", name=f"kt{i}")
                 for i in range(4)]
        va_sb = [qkpool.tile([128, H * 65], BF, tag=f"va{i}", name=f"va{i}")
                 for i in range(NB)]
        oT_sb = [qkpool.tile([128, N], BF, tag=f"ot{i}", name=f"ot{i}")
                 for i in range(4)]
        gsat_sb = qkpool.tile([64, N], BF, tag="gsat", name="gsat")
        warm_sb = qkpool.tile([1, 8], BF, tag="warm", name="warm")

        # pre-warm the exp activation table (one tiny call; ~2.7us table load)
        nc.scalar.activation(warm_sb[:, 0:4], bo_sb[0:1, 0:4],
                             mybir.ActivationFunctionType.Exp)

        # ---------- q/k/gsat projections ----------
        with tc.tile_pool(name="proj_psum", bufs=2, space="PSUM") as pp:
            for m in range(4):
                for c in range(2):
                    csl = slice(512 * c, 512 * c + 512)
                    pq = pp.tile([128, 512], F32, tag="pq")
                    pk = pp.tile([128, 512], F32, tag="pk")
                    for k in range(4):
                        msl = slice(128 * m, 128 * m + 128)
                        nc.tensor.matmul(pq, wq_sb[k][:, msl], xT_sb[k][:, csl],
                                         start=(k == 0), stop=(k == 3))
                        nc.tensor.matmul(pk, wk_sb[k][:, msl], xT_sb[k][:, csl],
                                         start=(k == 0), stop=(k == 3))
                    nc.scalar.copy(out=qT_sb[m][:, csl], in_=pq)
                    nc.scalar.copy(out=kT_sb[m][:, csl], in_=pk)
            for c in range(2):
                csl = slice(512 * c, 512 * c + 512)
                psg = pp.tile([16, 512], F32, tag="psg")
                for k in range(4):
                    nc.tensor.matmul(psg, ws_sb[k], xT_sb[k][:, csl],
                                     start=(k == 0), stop=(k == 3))
                # two copies at 32-row offsets so lo/hi sat matmuls can use
                # disjoint PE row groups and overlap
                nc.scalar.copy(out=gsat_sb[0:16, csl], in_=psg)
                nc.scalar.copy(out=gsat_sb[32:48, csl], in_=psg)

        # ---------- attention (+ v-projection interleaved with Gxr pair 0) ----
        with tc.tile_pool(name="pg_psum", bufs=2, space="PSUM") as pgp, \
                tc.tile_pool(name="st_psum", bufs=2, space="PSUM") as stp, \
                tc.tile_pool(name="o_psum", bufs=2, space="PSUM") as op, \
                tc.tile_pool(name="gxstage", bufs=1) as gxs, \
                tc.tile_pool(name="gpool", bufs=8) as gpool, \
                tc.tile_pool(name="es", bufs=2) as esp, \
                tc.tile_pool(name="small", bufs=2) as small:

            gtiles = {}    # (h, c) -> [128, 9, HALF] gathered pos^T bands

            def gx_steps(p, evac_cnt=[0]):
                """Generator of emission steps building pair p's Gxr tables.

                Each step: one 512-col chunk for BOTH heads (concurrent
                K=64 row-tiles) + PSUM evacuation; write DMAs fire per
                (head, ib); gathers per (head, query-half)."""
                h0 = 2 * p
                qTh = qT_sb[p]
                stg = {hx: gxs.tile([128, NB, WGX], BF, tag=f"stg{hx}",
                                    name=f"stg{2 * p + hx}") for hx in (0, 1)}
                gxr = {}
                for half in range(2):
                    for hx in (0, 1):
                        gxr[(hx, half)] = drampool.tile(
                            [HALF * PITCH], BF, tag="gx",
                            name=f"gxr{2 * p + hx}_{half}")

                for ib in range(NB):
                    isl = slice(128 * ib, 128 * ib + 128)
                    c_lo, c_hi = _valid_cols(ib)
                    chunks = list(range(c_lo, c_hi, 512))
                    for ci, c0 in enumerate(chunks):
                        cw = min(512, c_hi - c0)
                        last = ci == len(chunks) - 1

                        def step(ib=ib, isl=isl, c0=c0, cw=cw, last=last,
                                 c_lo=c_lo, c_hi=c_hi):
                            for hx in (0, 1):
                                hsl = slice(64 * hx, 64 * hx + 64)
                                pg = pgp.tile([128, 512], F32, tag="pg",
                                              name=f"pg{p}_{hx}_{ib}_{c0}")
                                nc.tensor.matmul(pg[:, :cw], qTh[hsl, isl],
                                                 etxr_sb[hsl, c0:c0 + cw],
                                                 start=True, stop=True)
                                evac_cnt[0] += 1
                                eng = (nc.scalar if evac_cnt[0] % 16 == 15
                                       else nc.vector)
                                if eng is nc.scalar:
                                    nc.scalar.copy(
                                        out=stg[hx][:, ib, c0:c0 + cw],
                                        in_=pg[:, :cw])
                                else:
                                    nc.vector.tensor_copy(
                                        stg[hx][:, ib, c0:c0 + cw],
                                        pg[:, :cw])
                            if last:
                                half = ib // 4
                                ibl = ib % 4
                                w = c_hi - c_lo
                                for hx in (0, 1):
                                    dst = _ap(gxr[(hx, half)],
                                              128 * ibl * PITCH + c_lo,
                                              [[PITCH, 128], [1, w]])
                                    nc.sync.dma_start(
                                        out=dst,
                                        in_=stg[hx][:, ib, c_lo:c_hi])
                        yield step

                    if ib % 4 == 3:
                        half = ib // 4

                        def gstep(half=half):
                            for hx in (0, 1):
                                g = gpool.tile([128, 9, HALF], BF, tag="g",
                                               name=f"g{2 * p + hx}_{half}")
                                src = _ap(gxr[(hx, half)], 0,
                                          [[128, HROWS], [1, ESIZE]])
                                nc.gpsimd.dma_gather(
                                    out_ap=g[:], in_ap=src,
                                    idxs_ap=idxs_sb[:, 32 * half:32 * half + 32],
                                    num_idxs=HALF, num_idxs_reg=HALF,
                                    elem_size=ESIZE, elem_step=128,
                                    transpose=True, single_packet=False,
                                )
                                gtiles[(2 * p + hx, half)] = g
                        yield gstep

            def v_steps():
                """v-projection + ones-augmented va build, as filler steps."""
                for nt in range(NB):
                    def step(nt=nt):
                        pv = pgp.tile([128, 512], F32, tag="pg",
                                      name=f"pv{nt}")
                        for k in range(4):
                            nsl = slice(128 * nt, 128 * nt + 128)
                            nc.tensor.matmul(pv, xT_sb[k][:, nsl], wv_sb[k],
                                             start=(k == 0), stop=(k == 3))
                        vout = _ap(va_sb[nt], 0,
                                   [[H * 65, 128], [65, H], [1, 64]])
                        vin = _ap(pv, 0, [[512, 128], [64, H], [1, 64]])
                        nc.vector.tensor_copy(vout, vin)
                        oc = _ap(va_sb[nt], 64, [[H * 65, 128], [65, H], [1, 1]])
                        nc.vector.memset(oc, 1.0)
                    yield step

            def drain(it, k=1):
                n = 0
                for s in it:
                    s()
                    n += 1
                    if n >= k:
                        return

            # interleave v-projection with pair-0 Gxr build
            gx0 = gx_steps(0)
            vp = v_steps()
            both = True
            while both:
                both = False
                for it in (gx0, vp, gx0):
                    for s in it:
                        s()
                        both = True
                        break

            zz = {}        # (h, c) -> Z row [1, 512]
            norm_q = []    # deferred normalization closures

            def make_norm(p, hx, c):
                def closure():
                    hsl = slice(64 * hx, 64 * hx + 64)
                    csl = slice(512 * c, 512 * c + 512)
                    bz = pgp.tile([128, 512], F32, tag="pg",
                                  name=f"bz{2 * p + hx}_{c}")
                    nc.tensor.matmul(bz, ones_sb,
                                     zz.pop((2 * p + hx, c)),
                                     start=True, stop=True)
                    rz = small.tile([128, 512], F32, tag=f"rz{hx}",
                                    name=f"rz{2 * p + hx}_{c}")
                    nc.vector.reciprocal_approx_fast(out=rz, in_=bz)
                    nc.vector.tensor_mul(oT_sb[p][hsl, csl],
                                         oT_sb[p][hsl, csl], rz[hsl, :])
                return closure

            def attention_pair(p, gx_iter):
                qTh = qT_sb[p]
                kTh = kT_sb[p]
                pend = None
                oacc_live = {}

                def emit_pv(c, rb, es2):
                    """PV accumulate; allocates oacc lazily at rb==0 so pool
                    rotation order matches instruction emission order."""
                    if rb == 0:
                        oacc_live[c] = {
                            hx: op.tile([65, 512], F32, tag="oacc",
                                        name=f"oacc{2 * p + hx}_{c}")
                            for hx in (0, 1)}
                    for hx in (0, 1):
                        h = 2 * p + hx
                        nc.tensor.matmul(
                            oacc_live[c][hx], va_sb[rb][:, 65 * h:65 * h + 65],
                            es2[hx], start=(rb == 0), stop=(rb == NB - 1))
                    if rb == NB - 1:
                        emit_stash(c, oacc_live.pop(c))

                def emit_stash(c, oacc2):
                    for hx in (0, 1):
                        h = 2 * p + hx
                        hsl = slice(64 * hx, 64 * hx + 64)
                        csl = slice(512 * c, 512 * c + 512)
                        z = small.tile([1, 512], BF, tag=f"zz{hx}",
                                       name=f"zz{h}_{c}")
                        nc.scalar.copy(out=z, in_=oacc2[hx][64:65, :])
                        zz[(h, c)] = z
                        nc.vector.tensor_copy(oT_sb[p][hsl, csl],
                                              oacc2[hx][0:64, :])
                        norm_q.append(make_norm(p, hx, c))

                for c in range(2):
                    c0a, c1a = 512 * c, 512 * c + 512
                    for rb in range(NB):
                        rsl = slice(128 * rb, 128 * rb + 128)
                        ib_lo, ib_hi = max(0, rb - 4), min(NB, rb + 5)
                        lo_end = 128 * max(0, rb - 4)
                        hi_st = 128 * min(NB, rb + 5)
                        bb_lo = max(ib_lo * 128, c0a)
                        bb_hi = min(ib_hi * 128, c1a)
                        nbi0 = (bb_hi - bb_lo) // 128 if bb_lo < bb_hi else 0

                        if pend is not None:
                            emit_pv(*pend)
                            pend = None

                        # Emission order keeps consecutive matmuls on
                        # disjoint PE row/col groups so they stream
                        # concurrently and LDWEIGHTS hides: kTq h0 (rows
                        # 0-63) || kTq h1 (64-127); sat lo (rows 0-15) ||
                        # sat hi (32-47); band quadrants (0-63 x lower out
                        # half) || (64-127 x upper).
                        n_lo = 1 if lo_end > c0a else 0
                        n_hi = 1 if hi_st < c1a else 0
                        nmem = 1 + n_lo + n_hi + (2 if nbi0 else 0)
                        cnt = [0, 0]
                        pst2 = {}
                        for hx in (0, 1):
                            h = 2 * p + hx
                            hsl = slice(64 * hx, 64 * hx + 64)
                            pst = stp.tile([128, 512], F32, tag=f"pst{hx}",
                                           name=f"pst{h}_{c}_{rb}")
                            pst2[hx] = pst
                            cnt[hx] += 1
                            nc.tensor.matmul(pst, kTh[hsl, rsl],
                                             qTh[hsl, c0a:c1a],
                                             start=True, stop=(nmem == 1))
                        for hx in (0, 1):
                            h = 2 * p + hx
                            if n_lo:
                                a, bnd = c0a, min(lo_end, c1a)
                                cnt[hx] += 1
                                nc.tensor.matmul(
                                    pst2[hx][:, a - c0a:bnd - c0a],
                                    sel_sb[0:16,
                                           128 * 2 * h:128 * 2 * h + 128],
                                    gsat_sb[0:16, a:bnd],
                                    start=False, stop=(cnt[hx] == nmem))
                            if n_hi:
                                a, bnd = max(hi_st, c0a), c1a
                                cnt[hx] += 1
                                nc.tensor.matmul(
                                    pst2[hx][:, a - c0a:bnd - c0a],
                                    sel_sb[32:48, 128 * (2 * h + 1):
                                           128 * (2 * h + 1) + 128],
                                    gsat_sb[32:48, a:bnd],
                                    start=False, stop=(cnt[hx] == nmem))
                        if nbi0:
                            ib0 = bb_lo // 128
                            q0 = rb - ib0 + 4
                            for hx, rh in ((0, 0), (1, 1), (1, 0), (0, 1)):
                                h = 2 * p + hx
                                g = gtiles[(h, c)]
                                ro = 64 * rh
                                rhs = _ap(g, ro * 9 * HALF + q0 * HALF
                                          + (bb_lo - c0a),
                                          [[9 * HALF, 64],
                                           [128 - HALF, nbi0], [1, 128]])
                                cnt[hx] += 1
                                nc.tensor.matmul(
                                    pst2[hx][ro:ro + 64,
                                             bb_lo - c0a:bb_hi - c0a],
                                    iden_sb[ro:ro + 64, ro:ro + 64], rhs,
                                    start=False, stop=(cnt[hx] == nmem),
                                    tile_position=(ro, ro))

                        if gx_iter is not None:
                            drain(gx_iter, 1)
                        if norm_q:
                            norm_q.pop(0)()

                        es2 = {}
                        for hx in (0, 1):
                            es = esp.tile([128, 512], BF, tag=f"es{hx}",
                                          name=f"es{2 * p + hx}_{c}_{rb}")
                            nc.scalar.activation(
                                es, pst2[hx],
                                mybir.ActivationFunctionType.Exp)
                            es2[hx] = es
                        pend = (c, rb, es2)

                    # free the g tiles of this (pair, c)
                    for hx in (0, 1):
                        gtiles.pop((2 * p + hx, c), None)

                # drain remaining gx steps, then flush the last PV + stash
                if gx_iter is not None:
                    for s in gx_iter:
                        s()
                emit_pv(*pend)

            for p in range(4):
                gx_iter = gx_steps(p + 1) if p + 1 < 4 else None
                attention_pair(p, gx_iter)
            while norm_q:
                norm_q.pop(0)()

        # ---------- output projection ----------
        with tc.tile_pool(name="oproj_psum", bufs=4, space="PSUM") as opp, \
                tc.tile_pool(name="osb", bufs=4) as osb:
            for m in range(4):
                msl = slice(128 * m, 128 * m + 128)
                for c in range(2):
                    csl = slice(512 * c, 512 * c + 512)
                    po = opp.tile([128, 512], F32, tag="po")
                    for k in range(4):
                        nc.tensor.matmul(po, wo_sb[k][:, msl], oT_sb[k][:, csl],
                                         start=(k == 0), stop=(k == 3))
                    ot = osb.tile([128, 512], F32, tag="otf")
                    nc.scalar.add(out=ot, in_=po, add=bo_sb[:, m:m + 1])
                    nc.sync.dma_start(out=out[msl, csl], in_=ot)
    nc.compile()
    return nc


_NC_CACHE = {}


def _get_nc():
    if "nc" not in _NC_CACHE:
        _NC_CACHE["nc"] = build_bass()
    return _NC_CACHE["nc"]


def _host_prep(x, Wq, Wkv, Wo, bo, E):
    u = np.clip(639 - (np.arange(WGX) - 1), -512, 512) + 512
    etxr = E[u].T.astype(bf16)                                   # (64, WGX)
    etxr = np.concatenate([etxr, etxr], axis=0)                  # dup rows
    Wqs = (Wq * SCALE).astype(np.float32)
    wsat = np.zeros((DIM, 16), np.float32)
    for h in range(H):
        wsat[:, 2 * h] = Wqs[:, 64 * h:64 * h + 64] @ E[0]
        wsat[:, 2 * h + 1] = Wqs[:, 64 * h:64 * h + 64] @ E[2 * MAX_POS]
    sel = np.zeros((64, 16 * 128), bf16)
    for t in range(16):
        sel[t, 128 * t:128 * t + 128] = 1.0
        sel[32 + t, 128 * t:128 * t + 128] = 1.0
    ii = np.arange(N)
    idx = (10 * ii + 1 + ii // 128 - np.where(ii >= HALF, IDX_REBASE, 0))
    idx = idx.astype(np.int16)
    idxs = np.zeros((16, N // 16), np.int16)
    idxs[ii % 16, ii // 16] = idx
    idxs = np.tile(idxs, (8, 1))                                 # (128, 64)
    common = {
        "wq": Wqs.astype(bf16),
        "wk": Wkv[:, :DIM].astype(bf16),
        "wv": Wkv[:, DIM:].astype(bf16),
        "wo": Wo.astype(bf16),
        "bo": np.ascontiguousarray(bo.reshape(4, 128).T.astype(np.float32)),
        "etxr": np.ascontiguousarray(etxr),
        "wsat": wsat.astype(bf16),
        "sel": sel,
        "iden": np.eye(128, dtype=bf16),
        "idxs": idxs,
    }
    in_maps = []
    for b in range(B):
        m = dict(common)
        m["xT"] = np.ascontiguousarray(x[b].T.astype(bf16))
        in_maps.append(m)
    return in_maps


def kernel(x, Wq, Wkv, Wo, bo, E):
    x, Wq, Wkv, Wo, bo, E = (np.asarray(a) for a in (x, Wq, Wkv, Wo, bo, E))
    nc = _get_nc()
    in_maps = _host_prep(x, Wq, Wkv, Wo, bo, E)
    res = run_bass_kernel_spmd(nc, in_maps, core_ids=list(range(B)))
    out = np.stack([np.asarray(res.results[b]["out"], dtype=np.float32).T
                    for b in range(B)])
    return out


if __name__ == "__main__":
    rng = np.random.default_rng(0)
    inputs = {
        "x": rng.standard_normal((B, N, DIM), dtype=np.float32),
        "Wq": rng.standard_normal((DIM, H * DH), dtype=np.float32) * DIM ** -0.5,
        "Wkv": rng.standard_normal((DIM, 2 * H * DH), dtype=np.float32) * DIM ** -0.5,
        "Wo": rng.standard_normal((H * DH, DIM), dtype=np.float32) * (H * DH) ** -0.5,
        "bo": np.zeros((DIM,), np.float32),
        "E": rng.standard_normal((2 * MAX_POS + 1, DH), dtype=np.float32),
    }
    o = kernel(**inputs)
    print("kernel ran, out shape", o.shape, "sample", o[0, 0, :4])


# revision 20
# speedup vs baseline: 1.4729x; 1.1317x over previous
"""Trainium2 Bass kernel for relative-position multi-head attention.

Problem: B=8, N=1024, DIM=512, H=8, DH=64, MAX_POS=512
  out = softmax(q k^T * s + pos) v @ Wo + bo,  pos[i,r] = q_i . E[clip(i-r)+512] * s

Sharding: data-parallel over batch, one batch element per NeuronCore (8 cores).

Per-core algorithm (transposed layouts, bf16 matmuls, f32 PSUM accum), v2:
  qT/kT  = (Wq*s)^T x^T, Wk^T x^T          (inner, N)
  va     = [x Wv | ones]                   (N, 65 per head) - PV lhsT + Z row
  gsat   = (x Wsat)^T                      (16, N) rows 2h/2h+1 = q_h.E[0|1024]
  Gxr[i,u] = q_i . E[clip(639-u)+512]      (reversed q.E table, per head,
     valid-band columns only) -> DRAM with row pitch 1281 so each banded
     128-chunk of pos^T is a 256B-aligned row
  dma_gather(transpose=True) per (head, query-half): g[rr, q, i] = pos^T[r, i]
  S^T(rb) psum = k_b^T q + wide banded identity-matmul + K=16 selector
  matmuls against gsat for saturated ranges; exp on ScalarE -> bf16 SBUF;
  O^T accumulated with ones-augmented V (row 64 = Z); deferred per-pair
  normalize by approx-reciprocal; out^T = Wo^T O^T + bo. Host transposes.

Heads are processed in PAIRS (2p, 2p+1): their q/k/E data live at SBUF
partitions 0-63 / 64-127, so the K=64 matmuls of the two heads target
disjoint PE row-groups and execute concurrently (auto row-tiling).
The Gxr build + gather for pair p+1 is interleaved chunk-by-chunk into
attention of pair p; normalization of pair p runs during pair p+1.
"""

import numpy as np
import ml_dtypes
import sys

sys.path.insert(0, "/opt/trn_rl_repo")

import concourse.bass as bass  # noqa: E402
import concourse.mybir as mybir  # noqa: E402
import concourse.tile as tile  # noqa: E402
from concourse import bacc  # noqa: E402
from concourse.bass_utils import run_bass_kernel_spmd  # noqa: E402

B, N, DIM = 8, 1024, 512
H, DH = 8, 64
MAX_POS = 512
SCALE = DH ** -0.5
NB = N // 128          # 8 seq blocks
WGX = 1281             # padded Etxr width: [pad | 1279 cols | pad]
PITCH = 1281           # Gxr DRAM row pitch (elements)
ESIZE = 9 * 128        # gather row: 9 blocks of 128
HALF = N // 2          # queries per gather half
HROWS = (HALF * PITCH - ESIZE) // 128 + 1  # in-bounds gather view rows (5116)
IDX_REBASE = 5124      # row offset of query 512's data in its half tile

bf16 = ml_dtypes.bfloat16
BF = mybir.dt.bfloat16
F32 = mybir.dt.float32
I16 = mybir.dt.int16


def _ap(base, rel_off, pattern):
    """Custom access pattern relative to a tile's base AP."""
    b = base[:]
    return bass.AP(tensor=b.tensor, offset=b.offset + rel_off, ap=pattern)


def _valid_cols(ib):
    """Valid Gxr column range [c_lo, c_hi) of the WGX table for query block ib."""
    lo_q = max(0, 4 - ib)
    hi_q = min(8, 11 - ib)
    return 1 + 128 * lo_q, 128 * (hi_q + 2)


def build_bass():
    nc = bacc.Bacc()

    xT = nc.declare_dram_parameter("xT", [DIM, N], BF, isOutput=False)
    wq = nc.declare_dram_parameter("wq", [DIM, DIM], BF, isOutput=False)
    wk = nc.declare_dram_parameter("wk", [DIM, DIM], BF, isOutput=False)
    wv = nc.declare_dram_parameter("wv", [DIM, DIM], BF, isOutput=False)
    wo = nc.declare_dram_parameter("wo", [DIM, DIM], BF, isOutput=False)
    bo = nc.declare_dram_parameter("bo", [128, 4], F32, isOutput=False)
    etxr = nc.declare_dram_parameter("etxr", [128, WGX], BF, isOutput=False)
    wsat = nc.declare_dram_parameter("wsat", [DIM, 16], BF, isOutput=False)
    sel = nc.declare_dram_parameter("sel", [64, 16 * 128], BF, isOutput=False)
    iden = nc.declare_dram_parameter("iden", [128, 128], BF, isOutput=False)
    idxs = nc.declare_dram_parameter("idxs", [128, N // 16], I16, isOutput=False)
    out = nc.declare_dram_parameter("out", [DIM, N], F32, isOutput=True)

    with tile.TileContext(nc) as tc, tc.tile_pool(name="consts", bufs=1) as consts, \
            tc.tile_pool(name="qk", bufs=1) as qkpool, \
            tc.tile_pool(name="dram", bufs=4, space="DRAM") as drampool:

        # ---------- load constants ----------
        xT_sb = [consts.tile([128, N], BF, tag=f"xt{i}", name=f"xt{i}")
                 for i in range(4)]
        wq_sb = [consts.tile([128, DIM], BF, tag=f"wq{i}", name=f"wq{i}")
                 for i in range(4)]
        wk_sb = [consts.tile([128, DIM], BF, tag=f"wk{i}", name=f"wk{i}")
                 for i in range(4)]
        wv_sb = [consts.tile([128, DIM], BF, tag=f"wv{i}", name=f"wv{i}")
                 for i in range(4)]
        wo_sb = [consts.tile([128, DIM], BF, tag=f"wo{i}", name=f"wo{i}")
                 for i in range(4)]
        ws_sb = [consts.tile([128, 16], BF, tag=f"ws{i}", name=f"ws{i}")
                 for i in range(4)]
        for k in range(4):
            sl = slice(128 * k, 128 * k + 128)
            nc.sync.dma_start(out=xT_sb[k], in_=xT[sl, :])
            nc.sync.dma_start(out=wq_sb[k], in_=wq[sl, :])
            nc.sync.dma_start(out=wk_sb[k], in_=wk[sl, :])
            nc.sync.dma_start(out=wv_sb[k], in_=wv[sl, :])
            nc.sync.dma_start(out=wo_sb[k], in_=wo[sl, :])
            nc.sync.dma_start(out=ws_sb[k], in_=wsat[sl, :])
        etxr_sb = consts.tile([128, WGX], BF)
        nc.sync.dma_start(out=etxr_sb, in_=etxr[:, :])
        sel_sb = consts.tile([64, 16 * 128], BF)
        nc.sync.dma_start(out=sel_sb, in_=sel[:, :])
        iden_sb = consts.tile([128, 128], BF)
        nc.sync.dma_start(out=iden_sb, in_=iden[:, :])
        idxs_sb = consts.tile([128, N // 16], I16)
        nc.sync.dma_start(out=idxs_sb, in_=idxs[:, :])
        bo_sb = consts.tile([128, 4], F32)
        nc.sync.dma_start(out=bo_sb, in_=bo[:, :])
        ones_sb = consts.tile([1, 128], BF)
        nc.vector.memset(ones_sb, 1.0)

        # long-lived activations
        qT_sb = [qkpool.tile([128, N], BF, tag=f"qt{i}", name=f"qt{i}")
                 for i in range(4)]
        kT_sb = [qkpool.tile([128, N], BF, tag=f"kt{i}", name=f"kt{i}")
                 for i in range(4)]
        va_sb = [qkpool.tile([128, H * 65], BF, tag=f"va{i}", name=f"va{i}")
                 for i in range(NB)]
        oT_sb = [qkpool.tile([128, N], BF, tag=f"ot{i}", name=f"ot{i}")
                 for i in range(4)]
        gsat_sb = qkpool.tile([64, N], BF, tag="gsat", name="gsat")
        warm_sb = qkpool.tile([1, 8], BF, tag="warm", name="warm")

        # pre-warm the exp activation table (one tiny call; ~2.7us table load)
        nc.scalar.activation(warm_sb[:, 0:4], bo_sb[0:1, 0:4],
                             mybir.ActivationFunctionType.Exp)

        # ---------- attention (projections interleaved with Gxr pair 0) ----
        with tc.tile_pool(name="pg_psum", bufs=2, space="PSUM") as pgp, \
                tc.tile_pool(name="st_psum", bufs=2, space="PSUM") as stp, \
                tc.tile_pool(name="o_psum", bufs=2, space="PSUM") as op, \
                tc.tile_pool(name="gxstage", bufs=1) as gxs, \
                tc.tile_pool(name="gpool", bufs=8) as gpool, \
                tc.tile_pool(name="es", bufs=2) as esp, \
                tc.tile_pool(name="small", bufs=2) as small:

            gtiles = {}    # (h, c) -> [128, 9, HALF] gathered pos^T bands

            def emit_proj(kind, m, c):
                """One q/k/gsat projection unit: 4 accumulating matmuls +
                PSUM evacuation, using the attention pools' banks."""
                csl = slice(512 * c, 512 * c + 512)
                if kind == 'q' or kind == 'k':
                    w_sb, dst = ((wq_sb, qT_sb) if kind == 'q'
                                 else (wk_sb, kT_sb))
                    tag = "pst0" if kind == 'q' else "pst1"
                    pr = stp.tile([128, 512], F32, tag=tag,
                                  name=f"p{kind}{m}_{c}")
                    msl = slice(128 * m, 128 * m + 128)
                    for k in range(4):
                        nc.tensor.matmul(pr, w_sb[k][:, msl],
                                         xT_sb[k][:, csl],
                                         start=(k == 0), stop=(k == 3))
                    nc.scalar.copy(out=dst[m][:, csl], in_=pr)
                else:
                    psg = op.tile([65, 512], F32, tag="oacc",
                                  name=f"psg{c}")
                    for k in range(4):
                        nc.tensor.matmul(psg[0:16, :], ws_sb[k],
                                         xT_sb[k][:, csl],
                                         start=(k == 0), stop=(k == 3))
                    # two copies at 32-row offsets so lo/hi sat matmuls can
                    # use disjoint PE row groups and overlap
                    nc.scalar.copy(out=gsat_sb[0:16, csl], in_=psg[0:16, :])
                    nc.scalar.copy(out=gsat_sb[32:48, csl], in_=psg[0:16, :])

            def gx_steps(p, evac_cnt=[0]):
                """Generator of emission steps building pair p's Gxr tables.

                Each step: one 512-col chunk for BOTH heads (concurrent
                K=64 row-tiles) + PSUM evacuation; write DMAs fire per
                (head, ib); gathers per (head, query-half)."""
                h0 = 2 * p
                qTh = qT_sb[p]
                stg = {hx: gxs.tile([128, NB, WGX], BF, tag=f"stg{hx}",
                                    name=f"stg{2 * p + hx}") for hx in (0, 1)}
                gxr = {}
                for half in range(2):
                    for hx in (0, 1):
                        gxr[(hx, half)] = drampool.tile(
                            [HALF * PITCH], BF, tag="gx",
                            name=f"gxr{2 * p + hx}_{half}")

                for ib in range(NB):
                    isl = slice(128 * ib, 128 * ib + 128)
                    c_lo, c_hi = _valid_cols(ib)
                    chunks = list(range(c_lo, c_hi, 512))
                    for ci, c0 in enumerate(chunks):
                        cw = min(512, c_hi - c0)
                        last = ci == len(chunks) - 1

                        def step(ib=ib, isl=isl, c0=c0, cw=cw, last=last,
                                 c_lo=c_lo, c_hi=c_hi):
                            for hx in (0, 1):
                                hsl = slice(64 * hx, 64 * hx + 64)
                                pg = pgp.tile([128, 512], F32, tag="pg",
                                              name=f"pg{p}_{hx}_{ib}_{c0}")
                                nc.tensor.matmul(pg[:, :cw], qTh[hsl, isl],
                                                 etxr_sb[hsl, c0:c0 + cw],
                                                 start=True, stop=True)
                                evac_cnt[0] += 1
                                eng = (nc.scalar if evac_cnt[0] % 16 == 15
                                       else nc.vector)
                                if eng is nc.scalar:
                                    nc.scalar.copy(
                                        out=stg[hx][:, ib, c0:c0 + cw],
                                        in_=pg[:, :cw])
                                else:
                                    nc.vector.tensor_copy(
                                        stg[hx][:, ib, c0:c0 + cw],
                                        pg[:, :cw])
                            if last:
                                half = ib // 4
                                ibl = ib % 4
                                w = c_hi - c_lo
                                for hx in (0, 1):
                                    dst = _ap(gxr[(hx, half)],
                                              128 * ibl * PITCH + c_lo,
                                              [[PITCH, 128], [1, w]])
                                    nc.sync.dma_start(
                                        out=dst,
                                        in_=stg[hx][:, ib, c_lo:c_hi])
                        yield step

                    if ib % 4 == 3:
                        half = ib // 4

                        def gstep(half=half):
                            for hx in (0, 1):
                                g = gpool.tile([128, 9, HALF], BF, tag="g",
                                               name=f"g{2 * p + hx}_{half}")
                                src = _ap(gxr[(hx, half)], 0,
                                          [[128, HROWS], [1, ESIZE]])
                                nc.gpsimd.dma_gather(
                                    out_ap=g[:], in_ap=src,
                                    idxs_ap=idxs_sb[:, 32 * half:32 * half + 32],
                                    num_idxs=HALF, num_idxs_reg=HALF,
                                    elem_size=ESIZE, elem_step=128,
                                    transpose=True, single_packet=False,
                                )
                                gtiles[(2 * p + hx, half)] = g
                        yield gstep

            def v_steps():
                """v-projection + ones-augmented va build, as filler steps."""
                for nt in range(NB):
                    def step(nt=nt):
                        pv = pgp.tile([128, 512], F32, tag="pg",
                                      name=f"pv{nt}")
                        for k in range(4):
                            nsl = slice(128 * nt, 128 * nt + 128)
                            nc.tensor.matmul(pv, xT_sb[k][:, nsl], wv_sb[k],
                                             start=(k == 0), stop=(k == 3))
                        vout = _ap(va_sb[nt], 0,
                                   [[H * 65, 128], [65, H], [1, 64]])
                        vin = _ap(pv, 0, [[512, 128], [64, H], [1, 64]])
                        nc.vector.tensor_copy(vout, vin)
                        oc = _ap(va_sb[nt], 64, [[H * 65, 128], [65, H], [1, 1]])
                        nc.vector.memset(oc, 1.0)
                    yield step

            def drain(it, k=1):
                n = 0
                for s in it:
                    s()
                    n += 1
                    if n >= k:
                        return

            # q-proj for pair 0 eagerly, then interleave the remaining
            # projections (+v) with the pair-0 Gxr build as PE filler
            for c in range(2):
                emit_proj('q', 0, c)
            units = []
            for m in range(1, 4):
                for c in range(2):
                    units.append(('q', m, c))
            units.append(('g', 0, 0))
            units.append(('g', 0, 1))
            for m in range(4):
                for c in range(2):
                    units.append(('k', m, c))
            vp = v_steps()
            gx0 = gx_steps(0)
            ui = iter(units)
            more = True
            while more:
                more = False
                s = next(gx0, None)
                if s is not None:
                    s()
                    more = True
                u = next(ui, None)
                if u is not None:
                    emit_proj(*u)
                    more = True
                else:
                    sv = next(vp, None)
                    if sv is not None:
                        sv()
                        more = True

            zz = {}        # (h, c) -> Z row [1, 512]
            norm_q = []    # deferred normalization closures

            def make_norm(p, hx, c):
                def closure():
                    hsl = slice(64 * hx, 64 * hx + 64)
                    csl = slice(512 * c, 512 * c + 512)
                    bz = pgp.tile([128, 512], F32, tag="pg",
                                  name=f"bz{2 * p + hx}_{c}")
                    nc.tensor.matmul(bz, ones_sb,
                                     zz.pop((2 * p + hx, c)),
                                     start=True, stop=True)
                    rz = small.tile([128, 512], F32, tag=f"rz{hx}",
                                    name=f"rz{2 * p + hx}_{c}")
                    nc.vector.reciprocal_approx_fast(out=rz, in_=bz)
                    nc.vector.tensor_mul(oT_sb[p][hsl, csl],
                                         oT_sb[p][hsl, csl], rz[hsl, :])
                return closure

            def attention_pair(p, gx_iter):
                qTh = qT_sb[p]
                kTh = kT_sb[p]
                pend = None
                oacc_live = {}

                def emit_pv(c, rb, es2):
                    """PV accumulate; allocates oacc lazily at rb==0 so pool
                    rotation order matches instruction emission order."""
                    if rb == 0:
                        oacc_live[c] = {
                            hx: op.tile([65, 512], F32, tag="oacc",
                                        name=f"oacc{2 * p + hx}_{c}")
                            for hx in (0, 1)}
                    for hx in (0, 1):
                        h = 2 * p + hx
                        nc.tensor.matmul(
                            oacc_live[c][hx], va_sb[rb][:, 65 * h:65 * h + 65],
                            es2[hx], start=(rb == 0), stop=(rb == NB - 1))
                    if rb == NB - 1:
                        emit_stash(c, oacc_live.pop(c))

                def emit_stash(c, oacc2):
                    for hx in (0, 1):
                        h = 2 * p + hx
                        hsl = slice(64 * hx, 64 * hx + 64)
                        csl = slice(512 * c, 512 * c + 512)
                        z = small.tile([1, 512], BF, tag=f"zz{hx}",
                                       name=f"zz{h}_{c}")
                        nc.scalar.copy(out=z, in_=oacc2[hx][64:65, :])
                        zz[(h, c)] = z
                        nc.vector.tensor_copy(oT_sb[p][hsl, csl],
                                              oacc2[hx][0:64, :])
                        norm_q.append(make_norm(p, hx, c))

                for c in range(2):
                    c0a, c1a = 512 * c, 512 * c + 512
                    for rb in range(NB):
                        rsl = slice(128 * rb, 128 * rb + 128)
                        ib_lo, ib_hi = max(0, rb - 4), min(NB, rb + 5)
                        lo_end = 128 * max(0, rb - 4)
                        hi_st = 128 * min(NB, rb + 5)
                        bb_lo = max(ib_lo * 128, c0a)
                        bb_hi = min(ib_hi * 128, c1a)
                        nbi0 = (bb_hi - bb_lo) // 128 if bb_lo < bb_hi else 0

                        if pend is not None:
                            emit_pv(*pend)
                            pend = None

                        # Emission order keeps consecutive matmuls on
                        # disjoint PE row/col groups so they stream
                        # concurrently and LDWEIGHTS hides: kTq h0 (rows
                        # 0-63) || kTq h1 (64-127); sat lo (rows 0-15) ||
                        # sat hi (32-47); band quadrants (0-63 x lower out
                        # half) || (64-127 x upper).
                        n_lo = 1 if lo_end > c0a else 0
                        n_hi = 1 if hi_st < c1a else 0
                        nmem = 1 + n_lo + n_hi + (2 if nbi0 else 0)
                        cnt = [0, 0]
                        pst2 = {}
                        for hx in (0, 1):
                            h = 2 * p + hx
                            hsl = slice(64 * hx, 64 * hx + 64)
                            pst = stp.tile([128, 512], F32, tag=f"pst{hx}",
                                           name=f"pst{h}_{c}_{rb}")
                            pst2[hx] = pst
                            cnt[hx] += 1
                            nc.tensor.matmul(pst, kTh[hsl, rsl],
                                             qTh[hsl, c0a:c1a],
                                             start=True, stop=(nmem == 1))
                        for hx in (0, 1):
                            h = 2 * p + hx
                            if n_lo:
                                a, bnd = c0a, min(lo_end, c1a)
                                cnt[hx] += 1
                                nc.tensor.matmul(
                                    pst2[hx][:, a - c0a:bnd - c0a],
                                    sel_sb[0:16,
                                           128 * 2 * h:128 * 2 * h + 128],
                                    gsat_sb[0:16, a:bnd],
                                    start=False, stop=(cnt[hx] == nmem))
                            if n_hi:
                                a, bnd = max(hi_st, c0a), c1a
                                cnt[hx] += 1
                                nc.tensor.matmul(
                                    pst2[hx][:, a - c0a:bnd - c0a],
                                    sel_sb[32:48, 128 * (2 * h + 1):
                                           128 * (2 * h + 1) + 128],
                                    gsat_sb[32:48, a:bnd],
                                    start=False, stop=(cnt[hx] == nmem))
                        if nbi0:
                            ib0 = bb_lo // 128
                            q0 = rb - ib0 + 4
                            for hx, rh in ((0, 0), (1, 1), (1, 0), (0, 1)):
                                h = 2 * p + hx
                                g = gtiles[(h, c)]
                                ro = 64 * rh
                                rhs = _ap(g, ro * 9 * HALF + q0 * HALF
                                          + (bb_lo - c0a),
                                          [[9 * HALF, 64],
                                           [128 - HALF, nbi0], [1, 128]])
                                cnt[hx] += 1
                                nc.tensor.matmul(
                                    pst2[hx][ro:ro + 64,
                                             bb_lo - c0a:bb_hi - c0a],
                                    iden_sb[ro:ro + 64, ro:ro + 64], rhs,
                                    start=False, stop=(cnt[hx] == nmem),
                                    tile_position=(ro, ro))

                        if gx_iter is not None:
                            drain(gx_iter, 2 if (c == 0 and rb < 3) else 1)
                        if norm_q:
                            norm_q.pop(0)()

                        es2 = {}
                        for hx in (0, 1):
                            es = esp.tile([128, 512], BF, tag=f"es{hx}",
                                          name=f"es{2 * p + hx}_{c}_{rb}")
                            nc.scalar.activation(
                                es, pst2[hx],
                                mybir.ActivationFunctionType.Exp)
                            es2[hx] = es
                        pend = (c, rb, es2)

                    # free the g tiles of this (pair, c)
                    for hx in (0, 1):
                        gtiles.pop((2 * p + hx, c), None)

                # drain remaining gx steps, then flush the last PV + stash
                if gx_iter is not None:
                    for s in gx_iter:
                        s()
                emit_pv(*pend)

            for p in range(4):
                gx_iter = gx_steps(p + 1) if p + 1 < 4 else None
                attention_pair(p, gx_iter)
            while norm_q:
                norm_q.pop(0)()

        # ---------- output projection ----------
        with tc.tile_pool(name="oproj_psum", bufs=4, space="PSUM") as opp, \
                tc.tile_pool(name="osb", bufs=4) as osb:
            for m in range(4):
                msl = slice(128 * m, 128 * m + 128)
                for c in range(2):
                    csl = slice(512 * c, 512 * c + 512)
                    po = opp.tile([128, 512], F32, tag="po")
                    for k in range(4):
                        nc.tensor.matmul(po, wo_sb[k][:, msl], oT_sb[k][:, csl],
                                         start=(k == 0), stop=(k == 3))
                    ot = osb.tile([128, 512], F32, tag="otf")
                    nc.scalar.add(out=ot, in_=po, add=bo_sb[:, m:m + 1])
                    nc.sync.dma_start(out=out[msl, csl], in_=ot)
    nc.compile()
    return nc


_NC_CACHE = {}


def _get_nc():
    if "nc" not in _NC_CACHE:
        _NC_CACHE["nc"] = build_bass()
    return _NC_CACHE["nc"]


def _host_prep(x, Wq, Wkv, Wo, bo, E):
    u = np.clip(639 - (np.arange(WGX) - 1), -512, 512) + 512
    etxr = E[u].T.astype(bf16)                                   # (64, WGX)
    etxr = np.concatenate([etxr, etxr], axis=0)                  # dup rows
    Wqs = (Wq * SCALE).astype(np.float32)
    wsat = np.zeros((DIM, 16), np.float32)
    for h in range(H):
        wsat[:, 2 * h] = Wqs[:, 64 * h:64 * h + 64] @ E[0]
        wsat[:, 2 * h + 1] = Wqs[:, 64 * h:64 * h + 64] @ E[2 * MAX_POS]
    sel = np.zeros((64, 16 * 128), bf16)
    for t in range(16):
        sel[t, 128 * t:128 * t + 128] = 1.0
        sel[32 + t, 128 * t:128 * t + 128] = 1.0
    ii = np.arange(N)
    idx = (10 * ii + 1 + ii // 128 - np.where(ii >= HALF, IDX_REBASE, 0))
    idx = idx.astype(np.int16)
    idxs = np.zeros((16, N // 16), np.int16)
    idxs[ii % 16, ii // 16] = idx
    idxs = np.tile(idxs, (8, 1))                                 # (128, 64)
    common = {
        "wq": Wqs.astype(bf16),
        "wk": Wkv[:, :DIM].astype(bf16),
        "wv": Wkv[:, DIM:].astype(bf16),
        "wo": Wo.astype(bf16),
        "bo": np.ascontiguousarray(bo.reshape(4, 128).T.astype(np.float32)),
        "etxr": np.ascontiguousarray(etxr),
        "wsat": wsat.astype(bf16),
        "sel": sel,
        "iden": np.eye(128, dtype=bf16),
        "idxs": idxs,
    }
    in_maps = []
    for b in range(B):
        m = dict(common)
        m["xT"] = np.ascontiguousarray(x[b].T.astype(bf16))
        in_maps.append(m)
    return in_maps


def kernel(x, Wq, Wkv, Wo, bo, E):
    x, Wq, Wkv, Wo, bo, E = (np.asarray(a) for a in (x, Wq, Wkv, Wo, bo, E))
    nc = _get_nc()
    in_maps = _host_prep(x, Wq, Wkv, Wo, bo, E)
    res = run_bass_kernel_spmd(nc, in_maps, core_ids=list(range(B)))
    out = np.stack([np.asarray(res.results[b]["out"], dtype=np.float32).T
                    for b in range(B)])
    return out


if __name__ == "__main__":
    rng = np.random.default_rng(0)
    inputs = {
        "x": rng.standard_normal((B, N, DIM), dtype=np.float32),
        "Wq": rng.standard_normal((DIM, H * DH), dtype=np.float32) * DIM ** -0.5,
        "Wkv": rng.standard_normal((DIM, 2 * H * DH), dtype=np.float32) * DIM ** -0.5,
        "Wo": rng.standard_normal((H * DH, DIM), dtype=np.float32) * (H * DH) ** -0.5,
        "bo": np.zeros((DIM,), np.float32),
        "E": rng.standard_normal((2 * MAX_POS + 1, DH), dtype=np.float32),
    }
    o = kernel(**inputs)
    print("kernel ran, out shape", o.shape, "sample", o[0, 0, :4])
